# revision 2
# baseline (speedup 1.0000x reference)
"""AFT-Full (Attention Free Transformer) Trainium2 kernel.

Problem: nn_AFT_Full (B=8, H=W=128, C=512, fp32 io).

    q = x @ Wq + bq ; k = x @ Wk + bk ; v = x @ Wv + bv        (per-token C x C)
    ew = exp(w[:H, :W])                                         [H, W]
    num = einsum('iw,bhwc->bhic', ew, exp(k) * v)
    den = einsum('iw,bhwc->bhic', ew, exp(k))
    y   = sigmoid(q) * num / den
    out = y @ Wo + bo

Distribution: pure data-parallel over B — one batch element per NeuronCore,
8 cores, no collectives.  Weights + the [128,128] position-bias slice are
replicated.

Per-core dataflow (per h; all matmuls bf16 with fp32 PSUM accumulation):
  - x^T for the h-row arrives pre-transposed from the host as [c, w] chunks so
    the C-contraction projections need no on-device transpose.
  - q,k,v natural [w, c] via 12 N=512 matmuls sharing the x^T stationary.
  - ek = exp(k) (ScalarE), ekv = ek*v (VectorE).
  - num/den natural [i, c] with exp(w^T) stationary: 2 N=512 matmuls.
  - sigmoid(q) = 0.5*(1 + tanh(q/2)): tanh lives in the same ACT table set as
    exp, so there is exactly one table load in the whole kernel.  The 0.5 is
    folded into the transpose identity.
  - y1 = num * reciprocal_approx_fast(den);  y = (tanh(q/2) + 1) * y1.
  - y^T via 4 identity matmuls (identity pre-scaled by 0.5), out = y^T.T @ Wo.

The per-h work is emitted as a 4-deep software pipeline (qkv | aft | transpose
| out-proj) so every TensorE instruction's producers are >= 1 iteration old.
"""

import sys

if "/opt/trn_rl_repo" not in sys.path:
    sys.path.insert(0, "/opt/trn_rl_repo")

import ml_dtypes
import numpy as np

import concourse.bass as bass  # noqa: F401  (registers AP machinery)
import concourse.mybir as mybir
import concourse.tile as tile
from concourse import bacc
from concourse.bass_utils import run_bass_kernel_spmd

BF16 = mybir.dt.bfloat16
F32 = mybir.dt.float32
AF = mybir.ActivationFunctionType
OP = mybir.AluOpType

B, H, W, C = 8, 128, 128, 512
G = 8            # h-rows per input DMA group
NG = H // G      # 16 groups
NM = C // 128    # 4 contraction chunks

LAST_EXEC_NS = None
_NC_CACHE = {}


def _build_nc(has_bias: bool):
    nc = bacc.Bacc(None, target_bir_lowering=False)

    xt_d = nc.dram_tensor("xt", [NG, NM, 128, G * 128], BF16, kind="ExternalInput")
    w_d = nc.dram_tensor("wqkvo", [128, 4, NM, 512], BF16, kind="ExternalInput")
    wt_d = nc.dram_tensor("wt", [128, 128], F32, kind="ExternalInput")
    id_d = nc.dram_tensor("ident", [128, 128], BF16, kind="ExternalInput")
    if has_bias:
        bq_d = nc.dram_tensor("bqf", [128, C], F32, kind="ExternalInput")
        bv_d = nc.dram_tensor("bvf", [128, C], F32, kind="ExternalInput")
        bo_d = nc.dram_tensor("bof", [128, C], F32, kind="ExternalInput")
    out_d = nc.dram_tensor("out", [H, 128, C], F32, kind="ExternalOutput")

    with tile.TileContext(nc) as tc:
        with (
            tc.tile_pool(name="const", bufs=1) as cpool,
            tc.tile_pool(name="xt", bufs=2) as xpool,
            tc.tile_pool(name="work", bufs=3) as wpool,
            tc.tile_pool(name="ps_qkv", bufs=4, space="PSUM") as ps_qkv,
            tc.tile_pool(name="ps_nd", bufs=2, space="PSUM") as ps_nd,
            tc.tile_pool(name="ps_yt", bufs=1, space="PSUM") as ps_yt,
            tc.tile_pool(name="ps_out", bufs=1, space="PSUM") as ps_out,
        ):
            # ---- constants ----
            w_sb = cpool.tile([128, 4, NM, 512], BF16)
            nc.sync.dma_start(w_sb[:], w_d[:])
            wtmp = cpool.tile([128, 128], F32)
            nc.sync.dma_start(wtmp[:], wt_d[:])
            ewt = cpool.tile([128, 128], BF16)
            nc.scalar.activation(ewt[:], wtmp[:], AF.Exp)
            id_sb = cpool.tile([128, 128], BF16)
            nc.sync.dma_start(id_sb[:], id_d[:])
            if has_bias:
                bq_sb = cpool.tile([128, C], F32)
                nc.sync.dma_start(bq_sb[:], bq_d[:])
                bv_sb = cpool.tile([128, C], F32)
                nc.sync.dma_start(bv_sb[:], bv_d[:])
                bo_sb = cpool.tile([128, C], F32)
                nc.sync.dma_start(bo_sb[:], bo_d[:])

            gx = {}       # g -> group input tile [128, NM, G*128]
            st = {}       # h -> per-h state tiles

            def load_group(g):
                gx[g] = xpool.tile([128, NM, G * 128], BF16, tag="gx", name="gx")
                nc.sync.dma_start(gx[g][:], xt_d[g].rearrange("m p r -> p m r"))

            def stage_a(h):
                """q/k/v projections + exp(k) + tanh(q/2) + ek*v."""
                g, hg = divmod(h, G)
                if hg == 4 and g + 1 < NG:
                    load_group(g + 1)
                s = st[h] = {}
                q_ps = ps_qkv.tile([128, 512], F32, tag="qkv", name="qkv")
                k_ps = ps_qkv.tile([128, 512], F32, tag="qkv", name="qkv")
                v_ps = ps_qkv.tile([128, 512], F32, tag="qkv", name="qkv")
                for m in range(NM):
                    lhsT = gx[g][:, m, hg * 128:(hg + 1) * 128]
                    nc.tensor.matmul(q_ps[:], lhsT, w_sb[:, 0, m, :],
                                     start=(m == 0), stop=(m == NM - 1))
                    nc.tensor.matmul(k_ps[:], lhsT, w_sb[:, 1, m, :],
                                     start=(m == 0), stop=(m == NM - 1))
                    nc.tensor.matmul(v_ps[:], lhsT, w_sb[:, 2, m, :],
                                     start=(m == 0), stop=(m == NM - 1))
                ek = s["ek"] = wpool.tile([128, 512], BF16, tag="ek", name="ek")
                nc.scalar.activation(ek[:], k_ps[:], AF.Exp)
                tq = s["tq"] = wpool.tile([128, 512], F32, tag="tq", name="tq")
                if has_bias:
                    qb = wpool.tile([128, 512], F32, tag="qb", name="qb")
                    nc.vector.tensor_add(out=qb[:], in0=q_ps[:], in1=bq_sb[:])
                    nc.scalar.activation(tq[:], qb[:], AF.Tanh, scale=0.5)
                else:
                    nc.scalar.activation(tq[:], q_ps[:], AF.Tanh, scale=0.5)
                ekv = s["ekv"] = wpool.tile([128, 512], BF16, tag="ekv", name="ekv")
                nc.vector.tensor_mul(out=ekv[:], in0=ek[:], in1=v_ps[:])

            def stage_b(h):
                """AFT mixing matmuls + gated ratio."""
                s = st[h]
                num_ps = ps_nd.tile([128, 512], F32, tag="nd", name="nd")
                den_ps = ps_nd.tile([128, 512], F32, tag="nd", name="nd")
                nc.tensor.matmul(num_ps[:], ewt[:], s["ekv"][:], start=True, stop=True)
                nc.tensor.matmul(den_ps[:], ewt[:], s["ek"][:], start=True, stop=True)
                r = wpool.tile([128, 512], F32, tag="r", name="r")
                nc.vector.reciprocal_approx_fast(out=r[:], in_=den_ps[:])
                y1 = wpool.tile([128, 512], F32, tag="y1", name="y1")
                nc.vector.tensor_mul(out=y1[:], in0=num_ps[:], in1=r[:])
                if has_bias:
                    nc.vector.tensor_add(out=y1[:], in0=y1[:], in1=bv_sb[:])
                y = s["y"] = wpool.tile([128, 512], BF16, tag="y", name="y")
                # y = (tanh(q/2) + 1) * num/den   (the 0.5 lives in id_sb)
                nc.vector.scalar_tensor_tensor(
                    out=y[:], in0=s["tq"][:], scalar=1.0, in1=y1[:],
                    op0=OP.add, op1=OP.mult)
                del s["ek"], s["ekv"], s["tq"]

            def stage_c(h):
                """Transpose y via identity matmuls; copy back to SBUF."""
                s = st[h]
                yt_ps = ps_yt.tile([128, 512], F32, tag="yt", name="yt")
                for m in range(NM):
                    nc.tensor.matmul(yt_ps[:, m * 128:(m + 1) * 128],
                                     s["y"][:, m * 128:(m + 1) * 128], id_sb[:],
                                     start=(m == 0), stop=(m == NM - 1))
                yt = s["yt"] = wpool.tile([128, 512], BF16, tag="yt_sb", name="yt_sb")
                nc.scalar.copy(yt[:], yt_ps[:])
                del s["y"]

            def stage_d(h):
                """Output projection + store."""
                s = st[h]
                o_ps = ps_out.tile([128, 512], F32, tag="op", name="op")
                for m in range(NM):
                    nc.tensor.matmul(o_ps[:], s["yt"][:, m * 128:(m + 1) * 128],
                                     w_sb[:, 3, m, :],
                                     start=(m == 0), stop=(m == NM - 1))
                o_sb = wpool.tile([128, 512], F32, tag="o_sb", name="o_sb")
                if has_bias:
                    nc.vector.tensor_add(out=o_sb[:], in0=o_ps[:], in1=bo_sb[:])
                else:
                    nc.scalar.copy(o_sb[:], o_ps[:])
                nc.sync.dma_start(out_d[h], o_sb[:])
                del st[h]

            load_group(0)
            for t in range(H + 3):
                if t < H:
                    stage_a(t)
                if 1 <= t < H + 1:
                    stage_b(t - 1)
                if 2 <= t < H + 2:
                    stage_c(t - 2)
                if 3 <= t < H + 3:
                    stage_d(t - 3)

    nc.compile()
    return nc


def _prep_core_input(xb):
    """x[b] [H, W, C] f32 -> [NG, NM, 128, G*128] bf16 pre-transposed layout."""
    a = xb.transpose(2, 0, 1)                    # [c, h, w]
    a = a.reshape(NM, 128, NG, G, W)             # [m, c_sub, g, hg, w]
    a = a.transpose(2, 0, 1, 3, 4)               # [g, m, c_sub, hg, w]
    a = a.reshape(NG, NM, 128, G * W)
    return np.ascontiguousarray(a).astype(ml_dtypes.bfloat16)


def kernel(x, Wq, bq, Wk, bk, Wv, bv, w, Wo, bo, _profile=False):
    global LAST_EXEC_NS
    x = np.asarray(x, dtype=np.float32)
    assert x.shape == (B, H, W, C), x.shape

    # bk cancels exactly in num/den; bq, bv, bo need extra work only if nonzero.
    has_bias = bool(np.any(np.asarray(bq)) or np.any(np.asarray(bv))
                    or np.any(np.asarray(bo)))

    if has_bias not in _NC_CACHE:
        _NC_CACHE[has_bias] = _build_nc(has_bias)
    nc = _NC_CACHE[has_bias]

    wq4 = np.stack([np.asarray(Wq), np.asarray(Wk), np.asarray(Wv),
                    np.asarray(Wo)]).astype(np.float32)   # [4, C, C]
    wq4 = wq4.reshape(4, NM, 128, C).transpose(2, 0, 1, 3)  # [c_sub, which, m, c]
    w_host = np.ascontiguousarray(wq4).astype(ml_dtypes.bfloat16)
    wt_host = np.ascontiguousarray(np.asarray(w)[:H, :W].T).astype(np.float32)
    id_host = (np.eye(128) * 0.5).astype(ml_dtypes.bfloat16)

    base = {"wqkvo": w_host, "wt": wt_host, "ident": id_host}
    if has_bias:
        base["bqf"] = np.ascontiguousarray(
            np.broadcast_to(np.asarray(bq, np.float32), (128, C)))
        base["bvf"] = np.ascontiguousarray(
            np.broadcast_to(np.asarray(bv, np.float32), (128, C)))
        base["bof"] = np.ascontiguousarray(
            np.broadcast_to(np.asarray(bo, np.float32), (128, C)))

    in_maps = [dict(base, xt=_prep_core_input(x[b])) for b in range(B)]

    res = run_bass_kernel_spmd(nc, in_maps, core_ids=list(range(B)),
                               trace=bool(_profile))
    LAST_EXEC_NS = res.exec_time_ns
    return np.stack([res.results[b]["out"] for b in range(B)]).astype(np.float32)
